# revision 16
# baseline (speedup 1.0000x reference)
"""Trainium2 Bass kernel for nn_DiffMPC2 (100-step diagonal-QP SGD recursion).

The reference iterates  u <- u - LR*(2*q*u + p)  100 times, i.e. the affine
per-element map  u <- a*u + b  with  a = 1 - 0.02*q,  b = -0.01*p.  Closed
form:  u_100 = P*u0 + S'*p,  P = a^100,  S' = (P-1)/(2q) in [-1, -0.4337).

v4 design (vs the f32 baseline at 44.4us):
  * All HBM traffic is bf16 (norm tolerance 2e-2; this scheme measures
    ~4e-3 end-to-end): per core 3 MB in + ~3 MB store-side instead of 8 MB.
    q is clamped to >= 1e-7 on the host (true S'(0) = -1 is recovered by
    the clamp anyway); p is host-halved so Sl may carry a factor 2.
  * ACT (f32 internal, natural_log_exp set): on every chunk
        L = Ln(1 - 0.02q),  P = Exp(100 L)   [f32 -- the P-1 cancellation]
    and on ACT-R chunks additionally  G = Ln(q),  R = Exp(-G) = 1/q [bf16].
  * DVE-R chunks compute 1/q themselves: uint16 magic seed on the bf16
    bits of q -- z0 = bits^-1(0xFEF3 - bits(q)) ~ -1/q via (q XOR 0xFFFF)
    - 0x010C on a uint16 alias of the q tile (the uint16 ADD saturates on
    this HW, so the subtract form is required) -- plus one Newton step in
    bf16 (z1 = (q*z0 + 2)*z0, ~0.4% rms).  The sign is absorbed by
    negating that chunk's Pm1 term.  This moves 2 ACT passes to ~1.75
    DVE passes on ~25% of elements, balancing the two engines.
  * DVE per chunk (bf16 2x/4x perf modes):
        Pb   = P * 1                  tensor_scalar f32->bf16   2x_2p
        Pm1b = P - (1+EPS)            tensor_scalar f32->bf16   2x_2p
        Sl2  = Pm1b * R               tensor_tensor = (P-1-EPS)/q       2x
        Slc  = max(Sl2, -2)           tensor_scalar                     4x
        m1   = Pb * u0                tensor_tensor                     2x
        m2   = Slc * p_half           tensor_tensor                     2x
    The max(-2) clamp replaces the baseline's Taylor/max branch: EPS
    biases Sl downward wherever the f32 noise of P-1 is amplified by 1/q,
    and the true 2*S' always exceeds -2, so clamping recovers those
    elements.
  * u = m1 + m2 on DVE (2x); stores stream from the SP HWDGE queue.

Sharding: pure data parallel, batch split across 8 cores; per core
131072 rows x 4 ctrl cols = 524288 elems as [128, 4096] bf16.  Inputs are
host-packed per partition as [q | p/2 | u0].  Raw bass (explicit
per-engine programs + semaphores).
"""

import sys

for _p in (
    "/root/.axon_site",
    "/root/.axon_site/_ro/trn_rl_repo",
    "/root/.axon_site/_ro/pypackages",
):
    if _p not in sys.path:
        sys.path.append(_p)

import numpy as np
import ml_dtypes

from concourse import bass, mybir
from concourse.bass_utils import run_bass_kernel_spmd

N_CORES = 8
B = 1048576
S_DIM = 12
C_DIM = 4
PARTS = 128
F_TOTAL = (B // N_CORES) * C_DIM // PARTS  # 4096
CHUNKS = [512, 1024, 1024, 1024, 512]
DVE_R_CHUNKS = {0}
assert sum(CHUNKS) == F_TOTAL
N_CHUNKS = len(CHUNKS)
OFFS = [sum(CHUNKS[:i]) for i in range(N_CHUNKS)]
F_MAX = max(CHUNKS)

EPS = 6e-6
ONE_EPS = float(np.float32(1.0 + EPS))
Q_MIN = 1e-7
SEED_SUB = 0x010C  # (q ^ 0xFFFF) - 0x010C == 0xFEF3 - bits(q)  ->  ~ -1/q

_nc_cache = None


def _build_bass():
    f32 = mybir.dt.float32
    bf16 = mybir.dt.bfloat16
    u16 = mybir.dt.uint16
    u8 = mybir.dt.uint8
    Alu = mybir.AluOpType
    Act = mybir.ActivationFunctionType

    nc = bass.Bass()

    xin = nc.declare_dram_parameter("xin", [PARTS, 3 * F_TOTAL], bf16, isOutput=False)
    uo = nc.declare_dram_parameter("uo", [PARTS, F_TOTAL], bf16, isOutput=True)
    xr = xin.ap().rearrange("p (j f) -> p j f", j=3)

    def sb(name, cols, dtype):
        h = nc.alloc_sbuf_tensor(name, [PARTS, cols], dtype)
        return h, h.ap()

    # Input tiles: one slot per chunk -- every input DMA issues immediately.
    txh, tx = zip(
        *[sb(f"tx{c}", 3 * CHUNKS[c], bf16) for c in range(N_CHUNKS)]
    )
    tx = [t.rearrange("p (j f) -> p j f", j=3) for t in tx]
    # uint16 aliases of the q slice of each DVE-R chunk's input tile (q is
    # the first CHUNKS[c]*2 bytes of the per-partition region).
    tqu = {}
    for c in DVE_R_CHUNKS:
        addr = nc.lookup_mloc(txh[c]).addr
        tqu[c] = nc.alloc_sbuf_tensor_at(
            f"tqu{c}", [PARTS, CHUNKS[c]], u16, offset=addr
        ).ap()
    # z0 seed scratch: bf16 + uint16 alias at the same bytes.
    _, tz0h = sb("tz0", F_MAX, bf16)
    z0_addr = nc.lookup_mloc(_).addr
    tz0u = nc.alloc_sbuf_tensor_at("tz0u", [PARTS, F_MAX], u16, offset=z0_addr).ap()
    tz0b = tz0h

    # Full-size handoff buffers: no slot-reuse waits anywhere.
    _, _pad3 = sb("pad3", F_MAX, bf16)
    _, tP = sb("tP", F_TOTAL, f32)
    _, tR = sb("tR", F_TOTAL, bf16)
    _, tm1 = sb("tm1", F_MAX, bf16)
    _, tm2 = sb("tm2", F_MAX, bf16)
    _, _pad2 = sb("pad2", F_MAX, bf16)
    _, tout = sb("tout", F_TOTAL, bf16)
    # Engine-local scratch (in-order reuse is safe).
    _, tL = sb("tL", F_MAX, f32)
    _, tG = sb("tG", F_MAX, f32)
    _, tPb = sb("tPb", F_MAX, bf16)  # padding: keeps downstream SBUF placement
    _, tPm = sb("tPm", F_MAX, bf16)
    _, tSl = sb("tSl", F_MAX, bf16)
    _, tSlc = sb("tSlc", F_MAX, bf16)
    _, tt_ = sb("tt_", F_MAX, bf16)

    s_inq = [nc.alloc_semaphore(f"s_inq{c}") for c in range(N_CHUNKS)]
    s_inpu = [nc.alloc_semaphore(f"s_inpu{c}") for c in range(N_CHUNKS)]

    # Cumulative count of ACT-R chunks up to and including c.
    nR = []
    r = 0
    for c in range(N_CHUNKS):
        if c not in DVE_R_CHUNKS:
            r += 1
        nR.append(r)

    with (
        nc.Block() as block,
        nc.semaphore("s_actp") as s_actp,
        nc.semaphore("s_actr") as s_actr,
        nc.semaphore("s_dve") as s_dve,
        nc.semaphore("s_out") as s_out,
    ):

        @block.sync
        def _(sp):
            def dma_q(c):
                sl = slice(OFFS[c], OFFS[c] + CHUNKS[c])
                sp.dma_start(out=tx[c][:, 0, :], in_=xr[:, 0, sl]).then_inc(
                    s_inq[c], 16
                )

            def dma_pu(c):
                sl = slice(OFFS[c], OFFS[c] + CHUNKS[c])
                sp.dma_start(out=tx[c][:, 1:3, :], in_=xr[:, 1:3, sl]).then_inc(
                    s_inpu[c], 16
                )

            dma_q(0)
            dma_q(1)
            for c in range(N_CHUNKS):
                dma_pu(c)
                if c + 2 < N_CHUNKS:
                    dma_q(c + 2)
            for c in range(N_CHUNKS):
                sp.wait_ge(s_dve, c + 1)
                sp.dma_start(
                    out=uo.ap()[:, OFFS[c] : OFFS[c] + CHUNKS[c]],
                    in_=tout[:, OFFS[c] : OFFS[c] + CHUNKS[c]],
                ).then_inc(s_out, 16)
            sp.wait_ge(s_out, 16 * N_CHUNKS)

        @block.scalar
        def _(act):
            # Warm the natural_log_exp table set during the first input DMA.
            act.activation(tL[:, :1], tG[:, :1], Act.Ln, bias=1.0, scale=0.0)
            for c in range(N_CHUNKS):
                w = CHUNKS[c]
                sl = slice(OFFS[c], OFFS[c] + w)
                tq = tx[c][:, 0, :]
                act.wait_ge(s_inq[c], 16)
                act.activation(tL[:, :w], tq, Act.Ln, bias=1.0, scale=-0.02)
                act.activation(
                    tP[:, sl], tL[:, :w], Act.Exp, bias=0.0, scale=100.0
                ).then_inc(s_actp, 1)
                if c not in DVE_R_CHUNKS:
                    act.activation(tG[:, :w], tq, Act.Ln, bias=0.0, scale=1.0)
                    act.activation(
                        tR[:, sl], tG[:, :w], Act.Exp, bias=0.0, scale=-1.0
                    ).then_inc(s_actr, 1)

        @block.vector
        def _(v):
            for c in range(N_CHUNKS):
                w = CHUNKS[c]
                sl = slice(OFFS[c], OFFS[c] + w)
                tp_ = tx[c][:, 1, :]
                tu = tx[c][:, 2, :]
                dve_r = c in DVE_R_CHUNKS
                if dve_r:
                    # R = -1/q from the uint16 magic seed + one NR step;
                    # runs as soon as q lands (concurrent read with ACT).
                    v.wait_ge(s_inq[c], 16)
                    v.tensor_scalar(
                        tz0u[:, :w], tqu[c], 0xFFFF, None, Alu.bitwise_xor
                    )
                    v.tensor_scalar(tz0u[:, :w], tz0u[:, :w], SEED_SUB, None, Alu.subtract)
                    v.tensor_tensor(
                        tt_[:, :w], tx[c][:, 0, :], tz0b[:, :w], Alu.mult
                    )
                    v.tensor_scalar(tt_[:, :w], tt_[:, :w], 2.0, None, Alu.add)
                    v.tensor_tensor(tR[:, sl], tt_[:, :w], tz0b[:, :w], Alu.mult)
                v.wait_ge(s_actp, c + 1)
                # Pre-R work first: Pm1 (f32 subtract, bf16 out) and m1.
                if dve_r:
                    # R is -1/q here: negate the Pm1 factor instead.
                    v.tensor_scalar(
                        tPm[:, :w], tP[:, sl], -1.0, ONE_EPS, Alu.mult, Alu.add
                    )
                else:
                    v.tensor_scalar(
                        tPm[:, :w], tP[:, sl], ONE_EPS, None, Alu.subtract
                    )
                v.wait_ge(s_inpu[c], 16)
                # m1 = (P + 0) * u0 = P*u0 (pre-R work)
                v.scalar_tensor_tensor(
                    tm1[:, :w], tP[:, sl], 0.0, tu, Alu.add, Alu.mult
                )
                if not dve_r:
                    v.wait_ge(s_actr, nR[c])
                # R-gated tail: Sl2 = Pm1 * (1/q), clamp, m2, out.
                v.tensor_tensor(tSl[:, :w], tPm[:, :w], tR[:, sl], Alu.mult)
                v.tensor_scalar(tSlc[:, :w], tSl[:, :w], -2.0, None, Alu.max)
                v.tensor_tensor(tm2[:, :w], tSlc[:, :w], tp_, Alu.mult)
                v.tensor_tensor(
                    tout[:, sl], tm1[:, :w], tm2[:, :w], Alu.add
                ).then_inc(s_dve, 1)

    return nc


def _get_nc():
    global _nc_cache
    if _nc_cache is None:
        _nc_cache = _build_bass()
    return _nc_cache


def _prep_in_maps(Q, p, u_init):
    bf = ml_dtypes.bfloat16
    q_u = (
        np.maximum(np.ascontiguousarray(Q[:, S_DIM:], dtype=np.float32), Q_MIN)
        .astype(bf)
        .reshape(N_CORES, PARTS, F_TOTAL)
    )
    p_u = (
        (0.5 * np.ascontiguousarray(p[:, S_DIM:], dtype=np.float32))
        .astype(bf)
        .reshape(N_CORES, PARTS, F_TOTAL)
    )
    u0 = (
        np.ascontiguousarray(u_init, dtype=np.float32)
        .astype(bf)
        .reshape(N_CORES, PARTS, F_TOTAL)
    )
    xin = np.concatenate([q_u, p_u, u0], axis=2)  # [8, 128, 3*F_TOTAL] bf16
    return [{"xin": xin[c]} for c in range(N_CORES)]


def kernel(x_init, Q, p, u_init):
    assert Q.shape == (B, S_DIM + C_DIM) and u_init.shape == (B, C_DIM)
    nc = _get_nc()
    in_maps = _prep_in_maps(Q, p, u_init)
    res = run_bass_kernel_spmd(nc, in_maps, list(range(N_CORES)))
    out = np.stack([np.asarray(res.results[c]["uo"]) for c in range(N_CORES)])
    return out.reshape(B, C_DIM).astype(np.float32)


# revision 17
# speedup vs baseline: 1.0243x; 1.0243x over previous
"""Trainium2 Bass kernel for nn_DiffMPC2 (100-step diagonal-QP SGD recursion).

The reference iterates  u <- u - LR*(2*q*u + p)  100 times, i.e. the affine
per-element map  u <- a*u + b  with  a = 1 - 0.02*q,  b = -0.01*p.  Closed
form:  u_100 = P*u0 + S'*p,  P = a^100,  S' = (P-1)/(2q) in [-1, -0.4337).

v4 design (vs the f32 baseline at 44.4us):
  * All HBM traffic is bf16 (norm tolerance 2e-2; this scheme measures
    ~4e-3 end-to-end): per core 3 MB in + ~3 MB store-side instead of 8 MB.
    q is clamped to >= 1e-7 on the host (true S'(0) = -1 is recovered by
    the clamp anyway); p is host-halved so Sl may carry a factor 2.
  * ACT (f32 internal, natural_log_exp set): on every chunk
        L = Ln(1 - 0.02q),  P = Exp(100 L)   [f32 -- the P-1 cancellation]
    and on ACT-R chunks additionally  G = Ln(q),  R = Exp(-G) = 1/q [bf16].
  * DVE-R chunks compute 1/q themselves: uint16 magic seed on the bf16
    bits of q -- z0 = bits^-1(0xFEF3 - bits(q)) ~ -1/q via (q XOR 0xFFFF)
    - 0x010C on a uint16 alias of the q tile (the uint16 ADD saturates on
    this HW, so the subtract form is required) -- plus one Newton step in
    bf16 (z1 = (q*z0 + 2)*z0, ~0.4% rms).  The sign is absorbed by
    negating that chunk's Pm1 term.  This moves 2 ACT passes to ~1.75
    DVE passes on ~25% of elements, balancing the two engines.
  * DVE per chunk (bf16 2x/4x perf modes):
        Pb   = P * 1                  tensor_scalar f32->bf16   2x_2p
        Pm1b = P - (1+EPS)            tensor_scalar f32->bf16   2x_2p
        Sl2  = Pm1b * R               tensor_tensor = (P-1-EPS)/q       2x
        Slc  = max(Sl2, -2)           tensor_scalar                     4x
        m1   = Pb * u0                tensor_tensor                     2x
        m2   = Slc * p_half           tensor_tensor                     2x
    The max(-2) clamp replaces the baseline's Taylor/max branch: EPS
    biases Sl downward wherever the f32 noise of P-1 is amplified by 1/q,
    and the true 2*S' always exceeds -2, so clamping recovers those
    elements.
  * u = m1 + m2 on DVE (2x); stores stream from the SP HWDGE queue.

Sharding: pure data parallel, batch split across 8 cores; per core
131072 rows x 4 ctrl cols = 524288 elems as [128, 4096] bf16.  Inputs are
host-packed per partition as [q | p/2 | u0].  Raw bass (explicit
per-engine programs + semaphores).
"""

import sys

for _p in (
    "/root/.axon_site",
    "/root/.axon_site/_ro/trn_rl_repo",
    "/root/.axon_site/_ro/pypackages",
):
    if _p not in sys.path:
        sys.path.append(_p)

import numpy as np
import ml_dtypes

from concourse import bass, mybir
from concourse.bass_utils import run_bass_kernel_spmd

N_CORES = 8
B = 1048576
S_DIM = 12
C_DIM = 4
PARTS = 128
F_TOTAL = (B // N_CORES) * C_DIM // PARTS  # 4096
CHUNKS = [512, 1024, 1024, 1024, 512]
DVE_R_CHUNKS = {0}
assert sum(CHUNKS) == F_TOTAL
N_CHUNKS = len(CHUNKS)
OFFS = [sum(CHUNKS[:i]) for i in range(N_CHUNKS)]
F_MAX = max(CHUNKS)

EPS = 6e-6
ONE_EPS = float(np.float32(1.0 + EPS))
Q_MIN = 1e-7
SEED_SUB = 0x010C  # (q ^ 0xFFFF) - 0x010C == 0xFEF3 - bits(q)  ->  ~ -1/q

_nc_cache = None


def _build_bass():
    f32 = mybir.dt.float32
    bf16 = mybir.dt.bfloat16
    u16 = mybir.dt.uint16
    u8 = mybir.dt.uint8
    Alu = mybir.AluOpType
    Act = mybir.ActivationFunctionType

    nc = bass.Bass()

    xin = nc.declare_dram_parameter("xin", [PARTS, 3 * F_TOTAL], bf16, isOutput=False)
    uo = nc.declare_dram_parameter("uo", [PARTS, F_TOTAL], bf16, isOutput=True)
    xr = xin.ap().rearrange("p (j f) -> p j f", j=3)

    def sb(name, cols, dtype):
        h = nc.alloc_sbuf_tensor(name, [PARTS, cols], dtype)
        return h, h.ap()

    # Input tiles: one slot per chunk -- every input DMA issues immediately.
    txh, tx = zip(
        *[sb(f"tx{c}", 3 * CHUNKS[c], bf16) for c in range(N_CHUNKS)]
    )
    tx = [t.rearrange("p (j f) -> p j f", j=3) for t in tx]
    # uint16 aliases of the q slice of each DVE-R chunk's input tile (q is
    # the first CHUNKS[c]*2 bytes of the per-partition region).
    tqu = {}
    for c in DVE_R_CHUNKS:
        addr = nc.lookup_mloc(txh[c]).addr
        tqu[c] = nc.alloc_sbuf_tensor_at(
            f"tqu{c}", [PARTS, CHUNKS[c]], u16, offset=addr
        ).ap()
    # z0 seed scratch: bf16 + uint16 alias at the same bytes.
    _, tz0h = sb("tz0", F_MAX, bf16)
    z0_addr = nc.lookup_mloc(_).addr
    tz0u = nc.alloc_sbuf_tensor_at("tz0u", [PARTS, F_MAX], u16, offset=z0_addr).ap()
    tz0b = tz0h

    # Full-size handoff buffers: no slot-reuse waits anywhere.
    _, tP = sb("tP", F_TOTAL, f32)
    _, tR = sb("tR", F_TOTAL, bf16)
    _, _pad4 = sb("pad4", F_MAX, bf16)
    _, tm1 = sb("tm1", F_MAX, bf16)
    _, tm2 = sb("tm2", F_MAX, bf16)
    _, _pad2 = sb("pad2", F_MAX, bf16)
    _, tout = sb("tout", F_TOTAL, bf16)
    # Engine-local scratch (in-order reuse is safe).
    _, tL = sb("tL", F_MAX, f32)
    _, tG = sb("tG", F_MAX, f32)
    _, tPb = sb("tPb", F_MAX, bf16)  # padding: keeps downstream SBUF placement
    _, tPm = sb("tPm", F_MAX, bf16)
    _, tSl = sb("tSl", F_MAX, bf16)
    _, tSlc = sb("tSlc", F_MAX, bf16)
    _, tt_ = sb("tt_", F_MAX, bf16)

    s_inq = [nc.alloc_semaphore(f"s_inq{c}") for c in range(N_CHUNKS)]
    s_inpu = [nc.alloc_semaphore(f"s_inpu{c}") for c in range(N_CHUNKS)]

    # Cumulative count of ACT-R chunks up to and including c.
    nR = []
    r = 0
    for c in range(N_CHUNKS):
        if c not in DVE_R_CHUNKS:
            r += 1
        nR.append(r)

    with (
        nc.Block() as block,
        nc.semaphore("s_actp") as s_actp,
        nc.semaphore("s_actr") as s_actr,
        nc.semaphore("s_dve") as s_dve,
        nc.semaphore("s_out") as s_out,
    ):

        @block.sync
        def _(sp):
            def dma_q(c):
                sl = slice(OFFS[c], OFFS[c] + CHUNKS[c])
                sp.dma_start(out=tx[c][:, 0, :], in_=xr[:, 0, sl]).then_inc(
                    s_inq[c], 16
                )

            def dma_pu(c):
                sl = slice(OFFS[c], OFFS[c] + CHUNKS[c])
                sp.dma_start(out=tx[c][:, 1:3, :], in_=xr[:, 1:3, sl]).then_inc(
                    s_inpu[c], 16
                )

            dma_q(0)
            dma_q(1)
            for c in range(N_CHUNKS):
                dma_pu(c)
                if c + 2 < N_CHUNKS:
                    dma_q(c + 2)
            for c in range(N_CHUNKS):
                sp.wait_ge(s_dve, c + 1)
                sp.dma_start(
                    out=uo.ap()[:, OFFS[c] : OFFS[c] + CHUNKS[c]],
                    in_=tout[:, OFFS[c] : OFFS[c] + CHUNKS[c]],
                ).then_inc(s_out, 16)
            sp.wait_ge(s_out, 16 * N_CHUNKS)

        @block.scalar
        def _(act):
            # Warm the natural_log_exp table set during the first input DMA.
            act.activation(tL[:, :1], tG[:, :1], Act.Ln, bias=1.0, scale=0.0)
            for c in range(N_CHUNKS):
                w = CHUNKS[c]
                sl = slice(OFFS[c], OFFS[c] + w)
                tq = tx[c][:, 0, :]
                act.wait_ge(s_inq[c], 16)
                act.activation(tL[:, :w], tq, Act.Ln, bias=1.0, scale=-0.02)
                act.activation(
                    tP[:, sl], tL[:, :w], Act.Exp, bias=0.0, scale=100.0
                ).then_inc(s_actp, 1)
                if c not in DVE_R_CHUNKS:
                    act.activation(tG[:, :w], tq, Act.Ln, bias=0.0, scale=1.0)
                    act.activation(
                        tR[:, sl], tG[:, :w], Act.Exp, bias=0.0, scale=-1.0
                    ).then_inc(s_actr, 1)

        @block.vector
        def _(v):
            for c in range(N_CHUNKS):
                w = CHUNKS[c]
                sl = slice(OFFS[c], OFFS[c] + w)
                tp_ = tx[c][:, 1, :]
                tu = tx[c][:, 2, :]
                dve_r = c in DVE_R_CHUNKS
                if dve_r:
                    # R = -1/q from the uint16 magic seed + one NR step;
                    # runs as soon as q lands (concurrent read with ACT).
                    v.wait_ge(s_inq[c], 16)
                    v.tensor_scalar(
                        tz0u[:, :w], tqu[c], 0xFFFF, None, Alu.bitwise_xor
                    )
                    v.tensor_scalar(tz0u[:, :w], tz0u[:, :w], SEED_SUB, None, Alu.subtract)
                    v.tensor_tensor(
                        tt_[:, :w], tx[c][:, 0, :], tz0b[:, :w], Alu.mult
                    )
                    v.tensor_scalar(tt_[:, :w], tt_[:, :w], 2.0, None, Alu.add)
                    v.tensor_tensor(tR[:, sl], tt_[:, :w], tz0b[:, :w], Alu.mult)
                v.wait_ge(s_actp, c + 1)
                # Pre-R work first: Pm1 (f32 subtract, bf16 out) and m1.
                if dve_r:
                    # R is -1/q here: negate the Pm1 factor instead.
                    v.tensor_scalar(
                        tPm[:, :w], tP[:, sl], -1.0, ONE_EPS, Alu.mult, Alu.add
                    )
                else:
                    v.tensor_scalar(
                        tPm[:, :w], tP[:, sl], ONE_EPS, None, Alu.subtract
                    )
                v.wait_ge(s_inpu[c], 16)
                # m1 = (P + 0) * u0 = P*u0 (pre-R work)
                v.scalar_tensor_tensor(
                    tm1[:, :w], tP[:, sl], 0.0, tu, Alu.add, Alu.mult
                )
                if not dve_r:
                    v.wait_ge(s_actr, nR[c])
                # R-gated tail: Sl2 = Pm1 * (1/q), clamp, m2, out.
                v.tensor_tensor(tSl[:, :w], tPm[:, :w], tR[:, sl], Alu.mult)
                v.tensor_scalar(tSlc[:, :w], tSl[:, :w], -2.0, None, Alu.max)
                v.tensor_tensor(tm2[:, :w], tSlc[:, :w], tp_, Alu.mult)
                v.tensor_tensor(
                    tout[:, sl], tm1[:, :w], tm2[:, :w], Alu.add
                ).then_inc(s_dve, 1)

    return nc


def _get_nc():
    global _nc_cache
    if _nc_cache is None:
        _nc_cache = _build_bass()
    return _nc_cache


def _prep_in_maps(Q, p, u_init):
    bf = ml_dtypes.bfloat16
    q_u = (
        np.maximum(np.ascontiguousarray(Q[:, S_DIM:], dtype=np.float32), Q_MIN)
        .astype(bf)
        .reshape(N_CORES, PARTS, F_TOTAL)
    )
    p_u = (
        (0.5 * np.ascontiguousarray(p[:, S_DIM:], dtype=np.float32))
        .astype(bf)
        .reshape(N_CORES, PARTS, F_TOTAL)
    )
    u0 = (
        np.ascontiguousarray(u_init, dtype=np.float32)
        .astype(bf)
        .reshape(N_CORES, PARTS, F_TOTAL)
    )
    xin = np.concatenate([q_u, p_u, u0], axis=2)  # [8, 128, 3*F_TOTAL] bf16
    return [{"xin": xin[c]} for c in range(N_CORES)]


def kernel(x_init, Q, p, u_init):
    assert Q.shape == (B, S_DIM + C_DIM) and u_init.shape == (B, C_DIM)
    nc = _get_nc()
    in_maps = _prep_in_maps(Q, p, u_init)
    res = run_bass_kernel_spmd(nc, in_maps, list(range(N_CORES)))
    out = np.stack([np.asarray(res.results[c]["uo"]) for c in range(N_CORES)])
    return out.reshape(B, C_DIM).astype(np.float32)
